# revision 8
# baseline (speedup 1.0000x reference)
"""Multi-head attention block on 8 Trainium2 NeuronCores.

Reference: x:(2,4096,1024) f32 -> qkv proj (16 heads x 64) -> softmax
attention -> out proj. Sharding: tensor-parallel over heads (2 heads/core)
for qkv+attention, on-device AllToAll reshard (one collective per batch, so
batch 0's exchange and fc hide under batch 1's attention), then
token-parallel fc; rank j owns sequence slice [j*512,(j+1)*512) of BOTH
batches and the host re-interleaves the 8 output slices.

Precision: matmuls run as float32r (fp32 rounded to 11 fraction bits, full
PE rate at moving dim >= 256), fp32 PSUM accumulation, exp on the activation
engine (~2 ULP). End-to-end absmax-relative error vs the fp32 reference is
~2.5e-4 (validated against a bit-exact numpy model of the f32r rounding).

Per-core structure (heads A=2c, B=2c+1):
- x loads are contiguous token-major [128 x 1024] tiles (strided
  feature-major DMA measured 6 GB/s vs 127 GB/s contiguous); X^T tiles are
  produced on-chip by PE transposes whose PSUM evacuation doubles as the
  f32r rounding copy.
- qkv: X^T x W -> Q^T/K^T head-stacked [128 x 4096] (head dims on
  partitions); V token-major via narrow matmuls; Q/K/V share one
  [128 x 1536] PSUM tile per 512-token chunk. Batch 1's qkv is emitted
  interleaved with batch 0's attention so the PE fills ACT-bound gaps.
- scores: S^T = K^T-tile vs Q^T-chunk with the two heads row-tiled into
  array rows 0-63/64-127 (concurrent K=64 matmuls); 3 k-tiles share one
  [128 x 1536] PSUM tile so each exp instruction runs 1536 elements/lane.
- attn@V: per k-tile a fused stationary [V_A | ones | V_B] block; head A
  uses cols 0:128 (rows 0-63 = O_A, 64-127 = den_A), head B cols 64:192
  (rows 0-63 = den_B, 64-127 = O_B). The ones columns ride along free
  (matmul cost is streaming cycles), so the softmax denominator costs no
  extra PE work and lands partition-aligned for reciprocal + multiply.
  attn@V for group g-1 is emitted between group g's score/exp pairs.
- The V projection bias folds into the fc bias on the host (exact:
  P @ (V + 1 b_v^T) = P@V + den b_v^T and softmax rows sum to one).
- fc: after the AllToAlls each core holds all 1024 dims of its 1024
  tokens; 8 K=128 matmuls per (128-token tile, 512-col chunk) plus a K=1
  ones-row matmul adding the effective bias inside the accumulation group.
"""
import sys
import types

sys.path.insert(0, "/opt/trn_rl_repo")

import numpy as np

# NTFF profile hook shim: antenv.axon_hooks is absent in this image; register
# the ctypes hook so trace=True yields exec_time_ns. Best-effort only.
try:
    import antenv  # noqa: F401

    if "antenv.axon_hooks" not in sys.modules:
        _hooks_mod = types.ModuleType("antenv.axon_hooks")
        _hooks_mod._hook = None

        def _set_hook(h):
            _hooks_mod._hook = h

        def _get_hook():
            return _hooks_mod._hook

        _hooks_mod.set_axon_ntff_profile_hook = _set_hook
        _hooks_mod.get_axon_ntff_profile_hook = _get_hook
        sys.modules["antenv.axon_hooks"] = _hooks_mod
        try:
            from trn_agent_boot.trn_boot import _ntff_profile_via_ctypes

            _set_hook(_ntff_profile_via_ctypes("/opt/axon/libaxon_pjrt.so"))
        except Exception:
            pass
except Exception:
    pass

import concourse.bass as bass  # noqa: F401
import concourse.mybir as mybir
import concourse.bacc as bacc
import concourse.tile as tile
from concourse import bass_utils
from concourse.masks import make_identity

try:
    bass_utils.upload_artifacts = lambda tmpdir: "local://skipped"
except Exception:
    pass

F32 = mybir.dt.float32
F32R = mybir.dt.float32r
EXP = mybir.ActivationFunctionType.Exp

N_CORES = 8
B, N, F = 2, 4096, 1024
H, D = 16, 64
HPC = H // N_CORES
SCALE = D ** -0.5
TPC = B * N // N_CORES      # tokens per core = 1024 (512 from each batch)
NCH = N // 512              # 8 token-chunks per batch
KT = N // 128               # 32 key tiles per batch
QC = N // 512               # 8 query chunks per batch
KT_GROUPS = [(s, min(3, KT - s)) for s in range(0, KT, 3)]
VW = 192                    # [V_A | ones | V_B] block width per k-tile

_compiled = None


def _build():
    nc = bacc.Bacc("TRN2", target_bir_lowering=False, debug=False,
                   num_devices=N_CORES)
    x = nc.dram_tensor("x", [B, N, F], F32, kind="ExternalInput")
    wq = nc.dram_tensor("wq", [F, 128], F32, kind="ExternalInput")
    wk = nc.dram_tensor("wk", [F, 128], F32, kind="ExternalInput")
    wv = nc.dram_tensor("wv", [F, 128], F32, kind="ExternalInput")
    bq = nc.dram_tensor("bq", [128, 1], F32, kind="ExternalInput")
    bk = nc.dram_tensor("bk", [128, 1], F32, kind="ExternalInput")
    wfc = nc.dram_tensor("wfc", [F, F], F32, kind="ExternalInput")
    bfc = nc.dram_tensor("bfc", [1, F], F32, kind="ExternalInput")
    out = nc.dram_tensor("out", [TPC, F], F32, kind="ExternalOutput")

    with tile.TileContext(nc) as tc:
        with tc.tile_pool(name="sbW", bufs=1) as sbW, \
             tc.tile_pool(name="sbQK", bufs=2) as sbQK, \
             tc.tile_pool(name="sbV", bufs=2) as sbV, \
             tc.tile_pool(name="sbXc", bufs=1) as sbXc, \
             tc.tile_pool(name="sbXt", bufs=8) as sbXt, \
             tc.tile_pool(name="sbWf", bufs=16) as sbWf, \
             tc.tile_pool(name="sbP", bufs=2) as sbP, \
             tc.tile_pool(name="sbO", bufs=1) as sbO, \
             tc.tile_pool(name="sbF", bufs=1) as sbF, \
             tc.tile_pool(name="big", bufs=2, space="PSUM") as big, \
             tc.tile_pool(name="op", bufs=2, space="PSUM") as op_pool, \
             tc.tile_pool(name="dram", bufs=1, space="DRAM") as dram:

            wqr = sbW.tile([128, F], F32R, tag="wqr", name="wqr")
            wkr = sbW.tile([128, F], F32R, tag="wkr", name="wkr")
            wvr = sbW.tile([128, F], F32R, tag="wvr", name="wvr")
            for name_t, dst in [(wq, wqr), (wk, wkr), (wv, wvr)]:
                raw = sbF.tile([128, F], F32, tag="lraw", name="lraw")
                for fb in range(8):
                    nc.sync.dma_start(raw[:, fb * 128:(fb + 1) * 128],
                                      name_t.ap()[fb * 128:(fb + 1) * 128, :])
                nc.vector.tensor_copy(dst[:], raw[:])

            bq_sb = sbW.tile([128, 1], F32, tag="bq_sb", name="bq_sb")
            bk_sb = sbW.tile([128, 1], F32, tag="bk_sb", name="bk_sb")
            nc.sync.dma_start(bq_sb[:], bq.ap())
            nc.sync.dma_start(bk_sb[:], bk.ap())
            bfc_raw = sbXc.tile([1, F], F32, tag="xc", name="bfc_raw")
            nc.sync.dma_start(bfc_raw[:], bfc.ap())
            bfc_r = sbW.tile([1, F], F32R, tag="bfc_r", name="bfc_r")
            nc.vector.tensor_copy(bfc_r[:], bfc_raw[:])

            ones_f = sbW.tile([128, 64], F32, tag="ones_f", name="ones_f")
            nc.vector.memset(ones_f[:], 1.0)
            ones_r = sbW.tile([128, 64], F32R, tag="ones_r", name="ones_r")
            nc.vector.tensor_copy(ones_r[:], ones_f[:])
            fco_f = sbW.tile([1, 128], F32, tag="fco_f", name="fco_f")
            nc.vector.memset(fco_f[:], 1.0)
            fco_r = sbW.tile([1, 128], F32R, tag="fco_r", name="fco_r")
            nc.vector.tensor_copy(fco_r[:], fco_f[:])
            ident = sbW.tile([128, 128], F32, tag="ident", name="ident")
            make_identity(nc, ident[:])

            a2a = []
            for bb in range(B):
                a2a.append((
                    dram.tile([N_CORES, 128, 512], F32, tag=f"a2a_in{bb}",
                              name=f"a2a_in{bb}"),
                    dram.tile([N_CORES, 128, 512], F32, tag=f"a2a_out{bb}",
                              name=f"a2a_out{bb}"),
                ))

            xv = x.ap()  # [B, N, F]

            def alloc_bstate():
                return {
                    "q2t": sbQK.tile([128, N], F32R, tag="q2t", name="q2t"),
                    "k2t": sbQK.tile([128, N], F32R, tag="k2t", name="k2t"),
                    "vab": sbV.tile([128, KT * VW], F32R, tag="vab",
                                    name="vab"),
                }

            def qkv_chunk(st, b, ch):
                q2t, k2t, vab = st["q2t"], st["k2t"], st["vab"]
                xtr = [sbXt.tile([128, 512], F32R, tag="xt", name="xt")
                       for _ in range(8)]
                for t4 in range(4):
                    xc = sbXc.tile([128, F], F32, tag="xc", name="xc")
                    nc.sync.dma_start(
                        xc[:], xv[b][ch * 512 + t4 * 128:
                                     ch * 512 + (t4 + 1) * 128, :])
                    tp = big.tile([128, 1536], F32, tag="big", name="big")
                    for fb in range(8):
                        nc.tensor.transpose(tp[:, fb * 128:(fb + 1) * 128],
                                            xc[:, fb * 128:(fb + 1) * 128],
                                            ident[:])
                    for fb in range(8):
                        nc.vector.tensor_copy(
                            xtr[fb][:, t4 * 128:(t4 + 1) * 128],
                            tp[:, fb * 128:(fb + 1) * 128])
                ps = big.tile([128, 1536], F32, tag="big", name="big")
                for fb in range(8):
                    nc.tensor.matmul(ps[:, 0:512],
                                     wqr[:, fb * 128:(fb + 1) * 128],
                                     xtr[fb][:], start=(fb == 0),
                                     stop=(fb == 7))
                for fb in range(8):
                    nc.tensor.matmul(ps[:, 512:1024],
                                     wkr[:, fb * 128:(fb + 1) * 128],
                                     xtr[fb][:], start=(fb == 0),
                                     stop=(fb == 7))
                for t4 in range(4):
                    off = 1024 + t4 * 128
                    for fb in range(8):
                        nc.tensor.matmul(ps[:, off:off + 128],
                                         xtr[fb][:, t4 * 128:(t4 + 1) * 128],
                                         wvr[:, fb * 128:(fb + 1) * 128],
                                         start=(fb == 0), stop=(fb == 7))
                nc.vector.tensor_scalar_add(q2t[:, ch * 512:(ch + 1) * 512],
                                            ps[:, 0:512], bq_sb[:])
                nc.vector.tensor_scalar_add(k2t[:, ch * 512:(ch + 1) * 512],
                                            ps[:, 512:1024], bk_sb[:])
                for t4 in range(4):
                    kt = ch * 4 + t4
                    off = 1024 + t4 * 128
                    v0 = kt * VW
                    nc.vector.tensor_copy(vab[:, v0:v0 + 64],
                                          ps[:, off:off + 64])
                    nc.vector.tensor_copy(vab[:, v0 + 128:v0 + 192],
                                          ps[:, off + 64:off + 128])
                    nc.vector.tensor_copy(vab[:, v0 + 64:v0 + 128], ones_r[:])

            def attn_qc(st, b, qc):
                q2t, k2t, vab = st["q2t"], st["k2t"], st["vab"]
                ov = {"A": op_pool.tile([128, 512], F32, tag="op", name="op"),
                      "B": op_pool.tile([128, 512], F32, tag="op", name="op")}
                qsl = slice(qc * 512, (qc + 1) * 512)

                def emit_av(pend):
                    for head, k0g, gleng, ptg in pend:
                        for j in range(gleng):
                            kt = k0g + j
                            if head == "A":
                                lhs = vab[:, kt * VW:kt * VW + 128]
                            else:
                                lhs = vab[:, kt * VW + 64:kt * VW + 192]
                            nc.tensor.matmul(ov[head][:], lhs,
                                             ptg[:, j * 512:(j + 1) * 512],
                                             start=(kt == 0),
                                             stop=(kt == KT - 1),
                                             skip_group_check=True)

                pend = []
                for (k0, glen) in KT_GROUPS:
                    newpend = []
                    for head, lo, hi, rpos in [("A", 0, 64, (0, 0)),
                                               ("B", 64, 128, (64, 0))]:
                        bg = big.tile([128, 1536], F32, tag="big", name="big")
                        for j in range(glen):
                            kt = k0 + j
                            nc.tensor.matmul(
                                bg[:, j * 512:(j + 1) * 512],
                                k2t[lo:hi, kt * 128:(kt + 1) * 128],
                                q2t[lo:hi, qsl],
                                start=True, stop=True, tile_position=rpos)
                        pt = sbP.tile([128, 1536], F32R, tag="pt", name="pt")
                        nc.scalar.activation(pt[:, 0:glen * 512],
                                             bg[:, 0:glen * 512], EXP,
                                             scale=SCALE)
                        newpend.append((head, k0, glen, pt))
                    emit_av(pend)
                    pend = newpend
                emit_av(pend)

                rc = sbO.tile([128, 512], F32, tag="rc", name="rc")
                on = sbO.tile([128, 512], F32, tag="on", name="on")
                nc.vector.reciprocal(rc[0:64, :], ov["A"][64:128, :])
                nc.vector.tensor_tensor(on[0:64, :], ov["A"][0:64, :],
                                        rc[0:64, :], mybir.AluOpType.mult)
                nc.vector.reciprocal(rc[64:128, :], ov["B"][0:64, :])
                nc.vector.tensor_tensor(on[64:128, :], ov["B"][64:128, :],
                                        rc[64:128, :], mybir.AluOpType.mult)
                # rank qc owns this 512-token slice of batch b
                nc.sync.dma_start(a2a[b][0][qc, :, :], on[:])

            st0 = alloc_bstate()
            for ch in range(NCH):
                qkv_chunk(st0, 0, ch)
            st1 = None
            for qc in range(QC):
                attn_qc(st0, 0, qc)
                if st1 is None:
                    st1 = alloc_bstate()
                qkv_chunk(st1, 1, qc)
            nc.gpsimd.collective_compute(
                "AllToAll", mybir.AluOpType.bypass,
                replica_groups=[list(range(N_CORES))],
                ins=[a2a[0][0].opt()], outs=[a2a[0][1].opt()])

            # fc weights: 16 [128 x 512] f32r tiles (loaded during attention)
            wfc_t = []
            for db in range(8):
                raw = sbXc.tile([128, F], F32, tag="xc", name="xc")
                nc.sync.dma_start(raw[:], wfc.ap()[db * 128:(db + 1) * 128, :])
                for chn in range(2):
                    wt = sbWf.tile([128, 512], F32R, tag="wfct", name="wfct")
                    nc.vector.tensor_copy(wt[:],
                                          raw[:, chn * 512:(chn + 1) * 512])
                    wfc_t.append(wt)

            for qc in range(QC):
                attn_qc(st1, 1, qc)
            nc.gpsimd.collective_compute(
                "AllToAll", mybir.AluOpType.bypass,
                replica_groups=[list(range(N_CORES))],
                ins=[a2a[1][0].opt()], outs=[a2a[1][1].opt()])

            for tt in range(TPC // 128):
                bb, t128 = divmod(tt, 4)   # rows 0-511 = batch 0 slice
                lraw = sbF.tile([128, 8 * 128], F32, tag="lraw", name="lraw")
                for db in range(8):
                    nc.sync.dma_start(
                        lraw[:, db * 128:(db + 1) * 128],
                        a2a[bb][1][db, :, t128 * 128:(t128 + 1) * 128])
                lbr = sbF.tile([128, 8 * 128], F32R, tag="lbr", name="lbr")
                nc.vector.tensor_copy(lbr[:], lraw[:])
                for chn in range(2):
                    fps = big.tile([128, 1536], F32, tag="big", name="big")
                    for db in range(8):
                        nc.tensor.matmul(
                            fps[:, 0:512], lbr[:, db * 128:(db + 1) * 128],
                            wfc_t[db * 2 + chn][:],
                            start=(db == 0), stop=False,
                            skip_group_check=True)
                    nc.tensor.matmul(fps[:, 0:512], fco_r[:],
                                     bfc_r[:, chn * 512:(chn + 1) * 512],
                                     start=False, stop=True,
                                     skip_group_check=True)
                    fev = sbO.tile([128, 512], F32, tag="on", name="fev")
                    nc.vector.tensor_copy(fev[:], fps[:, 0:512])
                    nc.sync.dma_start(
                        out.ap()[tt * 128:(tt + 1) * 128,
                                 chn * 512:(chn + 1) * 512], fev[:])

    nc.compile()
    return nc


def _get_compiled():
    global _compiled
    if _compiled is None:
        _compiled = _build()
    return _compiled


def _prep_inputs(x, W_qkv, b_qkv, W_fc, b_fc):
    x = np.ascontiguousarray(np.asarray(x, np.float32))
    W_qkv = np.asarray(W_qkv, np.float32)
    b_qkv = np.asarray(b_qkv, np.float32)
    W_fc = np.ascontiguousarray(np.asarray(W_fc, np.float32))
    b_fc = np.asarray(b_fc, np.float32)
    bfc_eff = (b_fc.astype(np.float64)
               + b_qkv[2 * F:3 * F].astype(np.float64)
               @ W_fc.astype(np.float64))
    bfc_eff = np.ascontiguousarray(bfc_eff.astype(np.float32).reshape(1, F))
    in_maps = []
    for c in range(N_CORES):
        h0 = c * HPC
        cols = np.concatenate([np.arange(h * D, (h + 1) * D)
                               for h in range(h0, h0 + HPC)])
        in_maps.append({
            "x": x,
            "wq": np.ascontiguousarray(W_qkv[:, cols]),
            "wk": np.ascontiguousarray(W_qkv[:, F + cols]),
            "wv": np.ascontiguousarray(W_qkv[:, 2 * F + cols]),
            "bq": np.ascontiguousarray(b_qkv[cols].reshape(128, 1)),
            "bk": np.ascontiguousarray(b_qkv[F + cols].reshape(128, 1)),
            "wfc": W_fc,
            "bfc": bfc_eff,
        })
    return in_maps


def run(inputs, trace=False):
    nc = _get_compiled()
    in_maps = _prep_inputs(**inputs)
    last_err = None
    for attempt in range(2):
        try:
            res = bass_utils.run_bass_kernel_spmd(
                nc, in_maps, core_ids=list(range(N_CORES)), trace=trace)
            break
        except Exception as e:  # transient NRT_EXEC_UNIT_UNRECOVERABLE
            last_err = e
            if attempt == 1:
                raise
    else:
        raise last_err
    # rank j's output rows: 0-511 = batch0 seq slice j, 512-1023 = batch1
    full = np.empty((B, N, F), np.float32)
    for c in range(N_CORES):
        o = res.results[c]["out"]
        full[0, c * 512:(c + 1) * 512] = o[0:512]
        full[1, c * 512:(c + 1) * 512] = o[512:1024]
    return full, res


def kernel(**inputs) -> np.ndarray:
    out, _ = run(inputs, trace=False)
    return out
